# revision 1
# baseline (speedup 1.0000x reference)
"""MoE top-2 routing kernel for 8 Trainium2 NeuronCores.

Strategy (expert parallelism per the sharding hint):
  Launch A (data-parallel gate): each core computes softmax gate + top-2
    combine weights for its 1024-token slice, on device (fp32 matmul for
    exact top-k selection).
  Host: builds per-expert token index lists from the combine weights
    (routing bookkeeping only - all math stays on device).
  Launch B (expert-parallel): core i owns expert i. Gathers its tokens'
    rows of x by index (indirect DMA), transposes on the PE, runs the
    grouped GEMM against its resident expert weight in fp32r, seeds PSUM
    with the expert bias via a K=1 ones-matmul, scales rows by the gate
    probability, and writes the compact [C, 2048] result.
  Host: scatter-adds each expert's compact output into the full [B, 2048]
    output (each token appears in exactly two experts' lists).
"""

import numpy as np

import concourse.bass as bass
import concourse.mybir as mybir
from concourse.bass_utils import run_bass_kernel_spmd
from concourse.masks import make_identity
from concourse.tile import TileContext

B = 8192
D = 2048
O = 2048
E = 8
P = 128
C = 2304  # per-expert token capacity (18 chunks of 128; actual max load 2193)
BS = B // E  # tokens per core in the gate launch

f32 = mybir.dt.float32
f32r = mybir.dt.float32r
i32 = mybir.dt.int32


MAXW = 1  # this walrus build accepts one sync-wait command per instruction
_wsctr = [0]


def split_excess_waits(nc):
    """Post-pass: any instruction carrying more than MAXW sem-waits gets the
    excess moved onto spliced same-engine NoOps just before it (same-engine
    ge-waits executed earlier are semantically identical)."""
    import bass_rust

    for f in nc.m.functions:
        for blk in f.blocks:
            out = []
            changed = False
            for inst in blk.instructions:
                si = inst.sync_info
                if si is not None and len(si.on_wait) > MAXW:
                    waits = list(si.on_wait)
                    excess, keep = waits[:-MAXW], waits[-MAXW:]
                    for i in range(0, len(excess), MAXW):
                        _wsctr[0] += 1
                        nop = bass_rust.InstNoOp(
                            name=f"WSPLIT-{_wsctr[0]}", ins=[], outs=[]
                        )
                        nop.engine = inst.engine
                        nop.sync_info = mybir.SyncInfo(
                            on_wait=excess[i : i + MAXW], on_update=[]
                        )
                        out.append(nop)
                    inst.sync_info = mybir.SyncInfo(
                        on_wait=keep, on_update=list(si.on_update)
                    )
                    changed = True
                out.append(inst)
            if changed:
                blk.instructions = out


def build_gate_kernel():
    """Per core: gate for its BS-token slice. In: xT slice [D, BS], W_g
    [D, E], b_g [E, 1]. Out: combine weights c [BS, E] (top-2 masked
    softmax probs, zeros elsewhere)."""
    nc = bass.Bass()
    xt = nc.dram_tensor("xt", [D, BS], f32, kind="ExternalInput")
    wg = nc.dram_tensor("wg", [D, E], f32, kind="ExternalInput")
    bg = nc.dram_tensor("bg", [E, 1], f32, kind="ExternalInput")
    cout = nc.dram_tensor("c", [BS, E], f32, kind="ExternalOutput")
    NB = BS // 512

    with TileContext(nc) as tc:
        with (
            tc.tile_pool(name="const", bufs=1) as cpool,
            tc.tile_pool(name="work", bufs=2) as wpool,
            tc.tile_pool(name="psum", bufs=2, space="PSUM") as ppool,
            tc.tile_pool(name="psumg", bufs=4, space="PSUM") as pgpool,
        ):
            ident = cpool.tile([P, P], f32)
            make_identity(nc, ident[:])
            wgs = cpool.tile([P, 16, E], f32)
            nc.sync.dma_start(
                out=wgs[:], in_=wg.rearrange("(kt p) e -> p kt e", p=P)
            )
            bgs = cpool.tile([E, 1], f32)
            nc.sync.dma_start(out=bgs[:], in_=bg[:, :])

            for bc in range(NB):
                xts = wpool.tile([P, 16, 512], f32, tag="xts")
                xt3 = xt[:, bc * 512 : (bc + 1) * 512].rearrange(
                    "(kt p) b -> p kt b", p=P
                )
                for k in range(16):
                    nc.sync.dma_start(out=xts[:, k, :], in_=xt3[:, k, :])
                gt_ps = ppool.tile([E, 512], f32, tag="gt_ps")
                for k in range(16):
                    nc.tensor.matmul(
                        gt_ps[:],
                        lhsT=wgs[:, k, :],
                        rhs=xts[:, k, :],
                        start=(k == 0),
                        stop=(k == 15),
                    )
                gt = wpool.tile([E, 512], f32, tag="gt")
                # copy out of PSUM and add the gate bias (per-partition here)
                nc.vector.tensor_scalar_add(gt[:], gt_ps[:], bgs[:, 0:1])

                for t in range(4):
                    g_ps = pgpool.tile([P, E], f32, tag="g_ps")
                    nc.tensor.transpose(
                        out=g_ps[:],
                        in_=gt[:, t * P : (t + 1) * P],
                        identity=ident[:E, :E],
                    )
                    g = wpool.tile([P, E], f32, tag="g")
                    nc.vector.tensor_copy(g[:], g_ps[:])
                    mx = wpool.tile([P, 8], f32, tag="mx")
                    nc.vector.max(out=mx[:], in_=g[:])
                    nc.vector.tensor_scalar(
                        g[:], g[:], mx[:, 0:1], None, op0=mybir.AluOpType.subtract
                    )
                    nc.scalar.activation(g[:], g[:], mybir.ActivationFunctionType.Exp)
                    s = wpool.tile([P, 1], f32, tag="s")
                    nc.vector.reduce_sum(out=s[:], in_=g[:], axis=mybir.AxisListType.X)
                    r = wpool.tile([P, 1], f32, tag="r")
                    nc.vector.reciprocal(r[:], s[:])
                    nc.vector.tensor_scalar_mul(g[:], g[:], r[:, 0:1])
                    # top-2 mask: prob >= (second largest prob)
                    mx2 = wpool.tile([P, 8], f32, tag="mx2")
                    nc.vector.max(out=mx2[:], in_=g[:])
                    msk = wpool.tile([P, E], f32, tag="msk")
                    nc.vector.tensor_scalar(
                        msk[:], g[:], mx2[:, 1:2], None, op0=mybir.AluOpType.is_ge
                    )
                    cc = wpool.tile([P, E], f32, tag="cc")
                    nc.vector.tensor_mul(cc[:], g[:], msk[:])
                    row0 = bc * 512 + t * P
                    nc.sync.dma_start(out=cout[row0 : row0 + P, :], in_=cc[:])
    split_excess_waits(nc)
    return nc


def build_expert_kernel():
    """Per core: one expert. Gather C token rows by index, transpose on PE,
    fp32r GEMM vs resident weight, +bias (K=1 ones-matmul PSUM seed),
    scale rows by gate prob. Out: compact y [C, O]."""
    nc = bass.Bass()
    x = nc.dram_tensor("x", [B, D], f32, kind="ExternalInput")
    w = nc.dram_tensor("w", [D, O], f32, kind="ExternalInput")
    bia = nc.dram_tensor("bias", [1, O], f32, kind="ExternalInput")
    idx = nc.dram_tensor("idx", [C, 1], i32, kind="ExternalInput")
    prob = nc.dram_tensor("prob", [P, C // P], f32, kind="ExternalInput")
    y = nc.dram_tensor("y", [C, O], f32, kind="ExternalOutput")
    NM = C // P

    with TileContext(nc) as tc:
        with (
            tc.tile_pool(name="const", bufs=1) as cpool,
            tc.tile_pool(name="gath", bufs=3) as gpool,
            tc.tile_pool(name="xtp", bufs=2) as xpool,
            tc.tile_pool(name="yout", bufs=2) as ypool,
            tc.tile_pool(name="pst", bufs=4, space="PSUM") as tpool,
            tc.tile_pool(name="psy", bufs=1, space="PSUM") as yppool,
        ):
            ident = cpool.tile([P, P], f32)
            make_identity(nc, ident[:])
            ones_f = cpool.tile([1, P], f32)
            nc.vector.memset(ones_f[:], 1.0)
            ones = cpool.tile([1, P], f32r)
            nc.vector.tensor_copy(ones[:], ones_f[:])
            bias_sb = cpool.tile([1, O], f32r)
            nc.sync.dma_start(out=bias_sb[:], in_=bia[:, :].bitcast(f32r))
            prob_sb = cpool.tile([P, NM], f32)
            nc.sync.dma_start(out=prob_sb[:], in_=prob[:, :])
            wsb = cpool.tile([P, 16, O], f32r)
            w3 = w.rearrange("(kt p) o -> p kt o", p=P).bitcast(f32r)
            for k in range(16):
                nc.sync.dma_start(out=wsb[:, k, :], in_=w3[:, k, :])

            for m in range(NM):
                it = gpool.tile([P, 1], i32, tag="it")
                nc.sync.dma_start(out=it[:], in_=idx[m * P : (m + 1) * P, :])
                xg = gpool.tile([P, D], f32, tag="xg")
                nc.gpsimd.indirect_dma_start(
                    out=xg[:],
                    out_offset=None,
                    in_=x[:],
                    in_offset=bass.IndirectOffsetOnAxis(ap=it[:, :1], axis=0),
                )
                xts = []
                for k in range(16):
                    t_ps = tpool.tile([P, P], f32, tag="t_ps")
                    nc.tensor.transpose(
                        out=t_ps[:], in_=xg[:, k * P : (k + 1) * P], identity=ident[:]
                    )
                    xt = xpool.tile([P, P], f32r, tag=f"xt{k}")
                    nc.vector.tensor_copy(xt[:], t_ps[:])
                    xts.append(xt)
                yps = []
                for o in range(4):
                    ypo = yppool.tile([P, 512], f32, tag=f"yps{o}", name=f"yps{o}")
                    yps.append(ypo)
                for o in range(4):
                    nc.tensor.matmul(
                        yps[o][:],
                        lhsT=ones[:, :],
                        rhs=bias_sb[:, o * 512 : (o + 1) * 512],
                        start=True,
                        stop=False,
                    )
                for k in range(16):
                    for o in range(4):
                        nc.tensor.matmul(
                            yps[o][:],
                            lhsT=xts[k][:],
                            rhs=wsb[:, k, o * 512 : (o + 1) * 512],
                            start=False,
                            stop=(k == 15),
                        )
                ysb = ypool.tile([P, O], f32, tag="ysb")
                for o in range(4):
                    nc.vector.tensor_scalar_mul(
                        ysb[:, o * 512 : (o + 1) * 512],
                        yps[o][:],
                        prob_sb[:, m : m + 1],
                    )
                for q in range(2):
                    cs = q * (O // 2)
                    ce = cs + O // 2
                    nc.sync.dma_start(
                        out=y[m * P : (m + 1) * P, cs:ce], in_=ysb[:, cs:ce]
                    )
    split_excess_waits(nc)
    return nc


_gate_nc = None
_exp_nc = None


def kernel(x, W_e, b_e, W_g, b_g):
    global _gate_nc, _exp_nc
    x = np.ascontiguousarray(np.asarray(x, dtype=np.float32))
    W_e = np.ascontiguousarray(np.asarray(W_e, dtype=np.float32))
    b_e = np.ascontiguousarray(np.asarray(b_e, dtype=np.float32))
    W_g = np.ascontiguousarray(np.asarray(W_g, dtype=np.float32))
    b_g = np.ascontiguousarray(np.asarray(b_g, dtype=np.float32))

    xT = np.ascontiguousarray(x.T)  # [D, B] layout prep for the gate GEMM
    if _gate_nc is None:
        _gate_nc = build_gate_kernel()
    in_maps = [
        {
            "xt": np.ascontiguousarray(xT[:, i * BS : (i + 1) * BS]),
            "wg": W_g,
            "bg": b_g.reshape(E, 1),
        }
        for i in range(E)
    ]
    res_a = run_bass_kernel_spmd(_gate_nc, in_maps, core_ids=list(range(8)))
    c_full = np.concatenate([r["c"] for r in res_a.results], axis=0)  # [B, E]

    # Host routing bookkeeping: per-expert index lists from device-computed c
    idx_list, prob_list, n_list = [], [], []
    for e in range(E):
        sel = np.nonzero(c_full[:, e] > 0.0)[0].astype(np.int32)
        n = len(sel)
        assert n <= C, f"expert {e} over capacity: {n} > {C}"
        idxp = np.zeros((C, 1), np.int32)
        idxp[:n, 0] = sel
        probp = np.zeros(C, np.float32)
        probp[:n] = c_full[sel, e]
        idx_list.append(idxp)
        prob_list.append(np.ascontiguousarray(probp.reshape(C // P, P).T))
        n_list.append(n)

    if _exp_nc is None:
        _exp_nc = build_expert_kernel()
    in_maps = [
        {
            "x": x,
            "w": np.ascontiguousarray(W_e[e]),
            "bias": b_e[e].reshape(1, O),
            "idx": idx_list[e],
            "prob": prob_list[e],
        }
        for e in range(E)
    ]
    res_b = run_bass_kernel_spmd(_exp_nc, in_maps, core_ids=list(range(8)))

    out = np.zeros((B, O), np.float32)
    for e in range(E):
        n = n_list[e]
        out[idx_list[e][:n, 0]] += res_b.results[e]["y"][:n]
    return out



# revision 4
# speedup vs baseline: 1.6603x; 1.6603x over previous
"""MoE top-2 routing kernel for 8 Trainium2 NeuronCores.

Strategy (expert parallelism per the sharding hint):
  Launch A (data-parallel gate): each core computes the softmax gate for its
    1024-token slice on device. The gating GEMM runs as a 3-pass bf16 split
    (xhi*Whi + xhi*Wlo + xlo*Whi) whose logit error (~2e-6) is far below the
    minimum rank-2/3 logit gap (7.3e-5), so top-2 selection matches an fp32
    reference exactly. Softmax (exp, sum, reciprocal, scale) runs on
    ACT/DVE; full per-token probabilities are written out.
  Host: routing bookkeeping only - builds per-expert token index lists from
    the device-computed probabilities, gathers/packs/casts the token rows
    into the fp8 operand planes, and scatter-adds the compact expert
    outputs into the final [B, O] buffer.
  Launch B (expert-parallel): core e owns expert e. The grouped GEMM runs
    on the PE in fp8-e4m3 DoubleRow mode (2 k-slices per pass, 0.5
    cycles/row) with a 3-product error-compensation scheme:
        y = x0@w0 + x0@w1 + x2@w0   where
        x0 = fp8(x), x2 = fp8(x - x0), w0 = fp8(64*W), w1 = fp8(64*W - w0)
    giving ~1e-3 relative error at 0.75 cycles/row (vs 1.0 for bf16).
    The expert bias is seeded into PSUM via a K=1 ones-matmul and the
    gate probability (with the 1/64 descale folded in) is applied by the
    scalar engine on the PSUM->SBUF copy.
"""

import numpy as np
import ml_dtypes

import concourse.bass as bass
import concourse.mybir as mybir
from concourse.bass import broadcast_tensor_aps
from concourse.bass_utils import run_bass_kernel_spmd
from concourse.masks import make_identity
from concourse.tile import TileContext

B = 8192
D = 2048
O = 2048
E = 8
P = 128
C = 2304  # per-expert token capacity (18 tiles of 128; actual max load 2193)
NM = C // P  # 18 m-tiles
BS = B // E  # 1024 tokens per core in the gate launch
NKT = D // P  # 16 k-tiles
NJ = D // 256  # 8 DoubleRow k-pairs
NCH = 4  # gate chunks per core
CH = BS // NCH  # 256 tokens per gate chunk
SW = 64.0  # weight pre-scale for the fp8 planes

f32 = mybir.dt.float32
bf16 = mybir.dt.bfloat16
f8 = mybir.dt.float8e4
i32 = mybir.dt.int32
FP8 = ml_dtypes.float8_e4m3
BF16 = ml_dtypes.bfloat16
DR = mybir.MatmulPerfMode.DoubleRow

MAXW = 1  # this walrus build accepts one sync-wait command per instruction
_wsctr = [0]


def split_excess_waits(nc):
    """Post-pass: any instruction carrying more than MAXW sem-waits gets the
    excess moved onto spliced same-engine NoOps just before it (same-engine
    ge-waits executed earlier are semantically identical)."""
    import bass_rust

    for f in nc.m.functions:
        for blk in f.blocks:
            out = []
            changed = False
            for inst in blk.instructions:
                si = inst.sync_info
                if si is not None and len(si.on_wait) > MAXW:
                    waits = list(si.on_wait)
                    excess, keep = waits[:-MAXW], waits[-MAXW:]
                    for i in range(0, len(excess), MAXW):
                        _wsctr[0] += 1
                        nop = bass_rust.InstNoOp(
                            name=f"WSPLIT-{_wsctr[0]}", ins=[], outs=[]
                        )
                        nop.engine = inst.engine
                        nop.sync_info = mybir.SyncInfo(
                            on_wait=excess[i : i + MAXW], on_update=[]
                        )
                        out.append(nop)
                    inst.sync_info = mybir.SyncInfo(
                        on_wait=keep, on_update=list(si.on_update)
                    )
                    changed = True
                out.append(inst)
            if changed:
                blk.instructions = out


def build_gate_kernel():
    """Per core: softmax gate probabilities for its BS-token slice.
    In: xhi/xlo [NKT, P, BS] bf16 (hi/lo split of x^T slice), wgh/wgl
    [NKT, P, E] bf16 (hi/lo split of W_g), bg [E, 1] f32.
    Out: ct [NCH, 2E, P] f32 - per chunk, transposed (chunk-expert, token)
    full softmax probabilities (no masking; host picks top-2)."""
    nc = bass.Bass()
    xh = nc.dram_tensor("xh", [NKT, P, BS], bf16, kind="ExternalInput")
    xl = nc.dram_tensor("xl", [NKT, P, BS], bf16, kind="ExternalInput")
    wgh = nc.dram_tensor("wgh", [NKT, P, E], bf16, kind="ExternalInput")
    wgl = nc.dram_tensor("wgl", [NKT, P, E], bf16, kind="ExternalInput")
    bg = nc.dram_tensor("bg", [E, 1], f32, kind="ExternalInput")
    ct = nc.dram_tensor("ct", [NCH, 2 * E, P], f32, kind="ExternalOutput")

    xh_r = xh.rearrange("k p b -> p k b")
    xl_r = xl.rearrange("k p b -> p k b")
    ct_r = ct.rearrange("c q t -> q c t")

    with TileContext(nc) as tc:
        with (
            tc.tile_pool(name="const", bufs=1) as cpool,
            tc.tile_pool(name="work", bufs=2) as wpool,
            tc.tile_pool(name="psl", bufs=2, space="PSUM") as lpool,
            tc.tile_pool(name="pst", bufs=2, space="PSUM") as tpool,
        ):
            ident = cpool.tile([P, P], f32)
            make_identity(nc, ident[:])
            wgh_s = cpool.tile([P, NKT, E], bf16)
            nc.sync.dma_start(out=wgh_s[:], in_=wgh.rearrange("k p e -> p k e"))
            wgl_s = cpool.tile([P, NKT, E], bf16)
            nc.sync.dma_start(out=wgl_s[:], in_=wgl.rearrange("k p e -> p k e"))
            bgs = cpool.tile([E, 1], f32)
            nc.sync.dma_start(out=bgs[:], in_=bg[:, :])

            for ch in range(NCH):
                s0 = ch * CH
                xhs = wpool.tile([P, NKT, CH], bf16, tag="xhs")
                nc.sync.dma_start(out=xhs[:], in_=xh_r[:, :, s0 : s0 + CH])
                xls = wpool.tile([P, NKT, CH], bf16, tag="xls")
                nc.sync.dma_start(out=xls[:], in_=xl_r[:, :, s0 : s0 + CH])

                gt_ps = lpool.tile([E, CH], f32, tag="gt")
                for k in range(NKT):
                    nc.tensor.matmul(
                        gt_ps[:], lhsT=wgh_s[:, k], rhs=xhs[:, k],
                        start=(k == 0), stop=False,
                    )
                for k in range(NKT):
                    nc.tensor.matmul(
                        gt_ps[:], lhsT=wgl_s[:, k], rhs=xhs[:, k],
                        start=False, stop=False,
                    )
                for k in range(NKT):
                    nc.tensor.matmul(
                        gt_ps[:], lhsT=wgh_s[:, k], rhs=xls[:, k],
                        start=False, stop=(k == NKT - 1),
                    )

                # exp(logit + bias) straight out of PSUM (bias per-partition)
                ex8 = wpool.tile([E, CH], f32, tag="ex8")
                nc.scalar.activation(
                    ex8[:], gt_ps[:], mybir.ActivationFunctionType.Exp,
                    bias=bgs[:, 0:1],
                )
                # transpose to token-major [P, 2, E]
                e4 = tpool.tile([P, 2, E], f32, tag="e4")
                for sc in range(2):
                    nc.tensor.transpose(
                        out=e4[:, sc, :], in_=ex8[:, sc * P : (sc + 1) * P],
                        identity=ident[:E, :E],
                    )
                s4 = wpool.tile([P, 2], f32, tag="s4")
                nc.vector.reduce_sum(out=s4[:], in_=e4[:], axis=mybir.AxisListType.X)
                r4 = wpool.tile([P, 2], f32, tag="r4")
                nc.vector.reciprocal(r4[:], s4[:])
                c4 = wpool.tile([P, 2 * E], f32, tag="c4")
                for sc in range(2):
                    e_ap, r_ap = broadcast_tensor_aps(
                        e4[:, sc, :], r4[:, sc : sc + 1]
                    )
                    nc.vector.tensor_mul(c4[:, sc * E : (sc + 1) * E], e_ap, r_ap)
                ct_ps = tpool.tile([2 * E, P], f32, tag="ctp")
                nc.tensor.transpose(out=ct_ps[:], in_=c4[:], identity=ident[:, :])
                ct_sb = wpool.tile([2 * E, P], f32, tag="ctsb")
                nc.vector.tensor_copy(ct_sb[:], ct_ps[:])
                nc.sync.dma_start(out=ct_r[:, ch, :], in_=ct_sb[:])
    split_excess_waits(nc)
    return nc


def build_expert_kernel():
    """Per core: one expert. Resident fp8 weight planes, fp8 DoubleRow
    grouped GEMM over pre-gathered/packed token planes, bias via K=1
    ones-matmul PSUM seed, prob scaling on the ACT PSUM->SBUF copy.
    Out: compact y [NM, P, O] bf16."""
    nc = bass.Bass()
    xt = nc.dram_tensor("xt", [NM, 2, P, D], f8, kind="ExternalInput")
    w = nc.dram_tensor("w", [2, NJ, P, 2, O], f8, kind="ExternalInput")
    bias = nc.dram_tensor("bias", [1, O], bf16, kind="ExternalInput")
    prob = nc.dram_tensor("prob", [P, NM], f32, kind="ExternalInput")
    y = nc.dram_tensor("y", [NM, P, O], bf16, kind="ExternalOutput")

    xt_r = xt.rearrange("m pl p (j i f) -> p m pl j i f", j=NJ, i=2)
    w_r = w.rearrange("pl j p i o -> p pl j i o")
    y_r = y.rearrange("m p o -> p m o")

    with TileContext(nc) as tc:
        with (
            tc.tile_pool(name="const", bufs=1) as cpool,
            tc.tile_pool(name="wts", bufs=1) as wtpool,
            tc.tile_pool(name="xin", bufs=3) as xpool,
            tc.tile_pool(name="yout", bufs=2) as ypool,
            tc.tile_pool(name="psy", bufs=2, space="PSUM") as ppool,
        ):
            ones_f = cpool.tile([1, P], f32)
            nc.vector.memset(ones_f[:], 1.0)
            ones = cpool.tile([1, P], bf16)
            nc.vector.tensor_copy(ones[:], ones_f[:])
            bias_sb = cpool.tile([1, O], bf16)
            nc.sync.dma_start(out=bias_sb[:], in_=bias[:, :])
            prob_sb = cpool.tile([P, NM], f32)
            nc.sync.dma_start(out=prob_sb[:], in_=prob[:, :])

            wt = [[None] * NJ for _ in range(2)]

            def load_w(pl, j):
                t = wtpool.tile([P, 2, O], f8, tag=f"w{pl}_{j}", name=f"w{pl}_{j}")
                nc.sync.dma_start(out=t[:], in_=w_r[:, pl, j])
                wt[pl][j] = t

            def load_x(m):
                t = xpool.tile([P, 2, NJ, 2, P], f8, tag="xt")
                nc.sync.dma_start(out=t[:], in_=xt_r[:, m])
                return t

            # j=0 weight planes + first two x tiles up front, then the rest
            load_w(0, 0)
            load_w(1, 0)
            xts = [load_x(0), load_x(1)]
            for j in range(1, NJ):
                load_w(0, j)
                load_w(1, j)

            for m in range(NM):
                if m + 2 < NM:
                    xts.append(load_x(m + 2))
                xtile = xts[m]
                ps = [
                    ppool.tile([P, 512], f32, tag=f"ps{c}", name=f"ps{c}")
                    for c in range(4)
                ]
                for c in range(4):
                    nc.tensor.matmul(
                        ps[c][:], lhsT=ones[:, :],
                        rhs=bias_sb[:, c * 512 : (c + 1) * 512],
                        start=True, stop=False,
                    )
                for j in range(NJ):
                    for xp, wp in ((0, 0), (0, 1), (1, 0)):
                        last = j == NJ - 1 and xp == 1
                        for c in range(4):
                            nc.tensor.matmul(
                                ps[c][:],
                                lhsT=xtile[:, xp, j],
                                rhs=wt[wp][j][:, :, c * 512 : (c + 1) * 512],
                                start=False, stop=last,
                                perf_mode=DR,
                            )
                ysb = ypool.tile([P, O], bf16, tag="ysb")
                for c in range(4):
                    nc.scalar.activation(
                        ysb[:, c * 512 : (c + 1) * 512], ps[c][:],
                        mybir.ActivationFunctionType.Copy,
                        scale=prob_sb[:, m : m + 1],
                    )
                nc.sync.dma_start(out=y_r[:, m], in_=ysb[:])
    split_excess_waits(nc)
    return nc


_gate_nc = None
_exp_nc = None


def kernel(x, W_e, b_e, W_g, b_g):
    global _gate_nc, _exp_nc
    x = np.ascontiguousarray(np.asarray(x, dtype=np.float32))
    W_e = np.asarray(W_e, dtype=np.float32)
    b_e = np.asarray(b_e, dtype=np.float32)
    W_g = np.asarray(W_g, dtype=np.float32)
    b_g = np.asarray(b_g, dtype=np.float32)

    # ---- Launch A: gate ----
    xT = np.ascontiguousarray(x.T)  # [D, B]
    xhi = xT.astype(BF16)
    xlo = (xT - xhi.astype(np.float32)).astype(BF16)
    wghi = W_g.astype(BF16)
    wglo = (W_g - wghi.astype(np.float32)).astype(BF16)
    wgh_d = np.ascontiguousarray(wghi.reshape(NKT, P, E))
    wgl_d = np.ascontiguousarray(wglo.reshape(NKT, P, E))
    bg_d = b_g.reshape(E, 1)

    if _gate_nc is None:
        _gate_nc = build_gate_kernel()
    in_maps = [
        {
            "xh": np.ascontiguousarray(
                xhi[:, i * BS : (i + 1) * BS].reshape(NKT, P, BS)
            ),
            "xl": np.ascontiguousarray(
                xlo[:, i * BS : (i + 1) * BS].reshape(NKT, P, BS)
            ),
            "wgh": wgh_d,
            "wgl": wgl_d,
            "bg": bg_d,
        }
        for i in range(E)
    ]
    res_a = run_bass_kernel_spmd(_gate_nc, in_maps, core_ids=list(range(8)))
    # ct [NCH, 2E, P] -> probs [BS, E] per core
    probs = np.concatenate(
        [
            r["ct"].reshape(NCH, 2, E, P).transpose(0, 1, 3, 2).reshape(BS, E)
            for r in res_a.results
        ],
        axis=0,
    )  # [B, E]

    # ---- Host routing bookkeeping ----
    top2 = np.argsort(-probs, axis=1, kind="stable")[:, :2]  # ties -> lower idx
    p2 = np.take_along_axis(probs, top2, axis=1)
    c_full = np.zeros_like(probs)
    np.put_along_axis(c_full, top2, p2, axis=1)

    # fp8 planes of x (computed once, rows gathered per expert)
    x0 = x.astype(FP8)
    x2 = (x - x0.astype(np.float32)).astype(FP8)

    idx_list, prob_list, n_list = [], [], []
    for e in range(E):
        sel = np.nonzero(c_full[:, e] > 0.0)[0].astype(np.int32)
        n = len(sel)
        assert n <= C, f"expert {e} over capacity: {n} > {C}"
        idxp = np.zeros(C, np.int32)
        idxp[:n] = sel
        probp = np.zeros(C, np.float32)
        probp[:n] = c_full[sel, e]
        idx_list.append(idxp)
        prob_list.append(np.ascontiguousarray((probp / SW).reshape(NM, P).T))
        n_list.append(n)

    def pack_x(plane, idxp):
        g = plane[idxp]  # [C, D] fp8
        return g.reshape(NM, P, NJ, 2, P).transpose(0, 4, 2, 3, 1)

    def pack_w(Wf):
        # [D, O] float -> [NJ, P, 2, O] fp8 plane pair
        Wp = np.clip(Wf * SW, -240, 240)
        w0 = Wp.astype(FP8)
        w1 = np.clip(Wp - w0.astype(np.float32), -240, 240).astype(FP8)
        return (
            w0.reshape(NJ, 2, P, O).transpose(0, 2, 1, 3),
            w1.reshape(NJ, 2, P, O).transpose(0, 2, 1, 3),
        )

    if _exp_nc is None:
        _exp_nc = build_expert_kernel()
    in_maps = []
    for e in range(E):
        xt_d = np.empty((NM, 2, P, D), FP8)
        xt_d[:, 0] = pack_x(x0, idx_list[e]).reshape(NM, P, D)
        xt_d[:, 1] = pack_x(x2, idx_list[e]).reshape(NM, P, D)
        w0_d, w1_d = pack_w(W_e[e])
        w_d = np.empty((2, NJ, P, 2, O), FP8)
        w_d[0] = w0_d
        w_d[1] = w1_d
        in_maps.append(
            {
                "xt": xt_d,
                "w": w_d,
                "bias": (b_e[e] * SW).astype(BF16).reshape(1, O),
                "prob": prob_list[e],
            }
        )
    res_b = run_bass_kernel_spmd(_exp_nc, in_maps, core_ids=list(range(8)))

    out = np.zeros((B, O), np.float32)
    for e in range(E):
        n = n_list[e]
        ye = res_b.results[e]["y"].reshape(C, O)[:n].astype(np.float32)
        out[idx_list[e][:n]] += ye
    return out


# revision 10
# speedup vs baseline: 1.7425x; 1.0495x over previous
"""MoE top-2 routing kernel for 8 Trainium2 NeuronCores.

Strategy (expert parallelism per the sharding hint):
  Launch A (data-parallel gate): each core computes the softmax gate for its
    1024-token slice on device. The gating GEMM runs as a 3-pass bf16 split
    (xhi*Whi + xhi*Wlo + xlo*Whi) whose logit error (~2e-6) is far below the
    minimum rank-2/3 logit gap (7.3e-5), so top-2 selection matches an fp32
    reference exactly. Softmax (exp, sum, reciprocal, scale) runs on
    ACT/DVE; full per-token probabilities are written out.
  Host: routing bookkeeping only - builds per-expert token index lists from
    the device-computed probabilities, gathers/packs/casts the token rows
    into the fp8 operand planes, and scatter-adds the compact expert
    outputs into the final [B, O] buffer.
  Launch B (expert-parallel): core e owns expert e. The grouped GEMM runs
    on the PE in fp8-e4m3 DoubleRow mode (2 k-slices per pass, 0.5
    cycles/row) with a 3-product error-compensation scheme:
        y = x0@w0 + x0@w1 + x2@w0   where
        x0 = fp8(x), x2 = fp8(x - x0), w0 = fp8(64*W), w1 = fp8(64*W - w0)
    giving ~1e-3 relative error at 0.75 cycles/row (vs 1.0 for bf16).
    The expert bias is seeded into PSUM via a K=1 ones-matmul and the
    gate probability (with the 1/64 descale folded in) is applied by the
    scalar engine on the PSUM->SBUF copy.
"""

import numpy as np
import ml_dtypes

import concourse.bass as bass
import concourse.mybir as mybir
from concourse.bass import broadcast_tensor_aps
from concourse.bass_utils import run_bass_kernel_spmd
from concourse.masks import make_identity
from concourse.tile import TileContext

B = 8192
D = 2048
O = 2048
E = 8
P = 128
C = 2304  # per-expert token capacity (18 tiles of 128; actual max load 2193)
NM = C // P  # 18 m-tiles
BS = B // E  # 1024 tokens per core in the gate launch
NKT = D // P  # 16 k-tiles
NJ = D // 256  # 8 DoubleRow k-pairs
NCH = 4  # gate chunks per core
CH = BS // NCH  # 256 tokens per gate chunk
SW = 64.0  # weight pre-scale for the fp8 planes

f32 = mybir.dt.float32
bf16 = mybir.dt.bfloat16
f8 = mybir.dt.float8e4
i32 = mybir.dt.int32
FP8 = ml_dtypes.float8_e4m3
BF16 = ml_dtypes.bfloat16
DR = mybir.MatmulPerfMode.DoubleRow

MAXW = 1  # this walrus build accepts one sync-wait command per instruction
_wsctr = [0]


def split_excess_waits(nc):
    """Post-pass: any instruction carrying more than MAXW sem-waits gets the
    excess moved onto spliced same-engine NoOps just before it (same-engine
    ge-waits executed earlier are semantically identical)."""
    import bass_rust

    for f in nc.m.functions:
        for blk in f.blocks:
            out = []
            changed = False
            for inst in blk.instructions:
                si = inst.sync_info
                if si is not None and len(si.on_wait) > MAXW:
                    waits = list(si.on_wait)
                    excess, keep = waits[:-MAXW], waits[-MAXW:]
                    for i in range(0, len(excess), MAXW):
                        _wsctr[0] += 1
                        nop = bass_rust.InstNoOp(
                            name=f"WSPLIT-{_wsctr[0]}", ins=[], outs=[]
                        )
                        nop.engine = inst.engine
                        nop.sync_info = mybir.SyncInfo(
                            on_wait=excess[i : i + MAXW], on_update=[]
                        )
                        out.append(nop)
                    inst.sync_info = mybir.SyncInfo(
                        on_wait=keep, on_update=list(si.on_update)
                    )
                    changed = True
                out.append(inst)
            if changed:
                blk.instructions = out


def build_gate_kernel():
    """Per core: softmax gate probabilities for its BS-token slice.
    In: xhi/xlo [NKT, P, BS] bf16 (hi/lo split of x^T slice), wgh/wgl
    [NKT, P, E] bf16 (hi/lo split of W_g), bg [E, 1] f32.
    Out: ct [NCH, 2E, P] f32 - per chunk, transposed (chunk-expert, token)
    full softmax probabilities (no masking; host picks top-2)."""
    nc = bass.Bass()
    xh = nc.dram_tensor("xh", [NKT, P, BS], bf16, kind="ExternalInput")
    xl = nc.dram_tensor("xl", [NKT, P, BS], bf16, kind="ExternalInput")
    wgh = nc.dram_tensor("wgh", [NKT, P, E], bf16, kind="ExternalInput")
    wgl = nc.dram_tensor("wgl", [NKT, P, E], bf16, kind="ExternalInput")
    bg = nc.dram_tensor("bg", [E, 1], f32, kind="ExternalInput")
    ct = nc.dram_tensor("ct", [NCH, 2 * E, P], f32, kind="ExternalOutput")

    xh_r = xh.rearrange("k p b -> p k b")
    xl_r = xl.rearrange("k p b -> p k b")
    ct_r = ct.rearrange("c q t -> q c t")

    with TileContext(nc) as tc:
        with (
            tc.tile_pool(name="const", bufs=1) as cpool,
            tc.tile_pool(name="xing", bufs=4) as xgpool,
            tc.tile_pool(name="work", bufs=2) as wpool,
            tc.tile_pool(name="psl", bufs=2, space="PSUM") as lpool,
            tc.tile_pool(name="pst", bufs=2, space="PSUM") as tpool,
        ):
            ident = cpool.tile([P, P], f32)
            make_identity(nc, ident[:])
            wgh_s = cpool.tile([P, NKT, E], bf16)
            nc.sync.dma_start(out=wgh_s[:], in_=wgh.rearrange("k p e -> p k e"))
            wgl_s = cpool.tile([P, NKT, E], bf16)
            nc.sync.dma_start(out=wgl_s[:], in_=wgl.rearrange("k p e -> p k e"))
            bgs = cpool.tile([E, 1], f32)
            nc.sync.dma_start(out=bgs[:], in_=bg[:, :])

            for ch in range(NCH):
                s0 = ch * CH
                xhs = xgpool.tile([P, NKT, CH], bf16, tag="xhs")
                nc.sync.dma_start(out=xhs[:], in_=xh_r[:, :, s0 : s0 + CH])
                xls = xgpool.tile([P, NKT, CH], bf16, tag="xls")
                nc.sync.dma_start(out=xls[:], in_=xl_r[:, :, s0 : s0 + CH])

                gt_ps = lpool.tile([E, CH], f32, tag="gt")
                for k in range(NKT):
                    nc.tensor.matmul(
                        gt_ps[:], lhsT=wgh_s[:, k], rhs=xhs[:, k],
                        start=(k == 0), stop=False,
                    )
                for k in range(NKT):
                    nc.tensor.matmul(
                        gt_ps[:], lhsT=wgl_s[:, k], rhs=xhs[:, k],
                        start=False, stop=False,
                    )
                for k in range(NKT):
                    nc.tensor.matmul(
                        gt_ps[:], lhsT=wgh_s[:, k], rhs=xls[:, k],
                        start=False, stop=(k == NKT - 1),
                    )

                # exp(logit + bias) straight out of PSUM (bias per-partition)
                ex8 = wpool.tile([E, CH], f32, tag="ex8")
                nc.scalar.activation(
                    ex8[:], gt_ps[:], mybir.ActivationFunctionType.Exp,
                    bias=bgs[:, 0:1],
                )
                # transpose to token-major [P, 2, E]
                e4 = tpool.tile([P, 2, E], f32, tag="e4")
                for sc in range(2):
                    nc.tensor.transpose(
                        out=e4[:, sc, :], in_=ex8[:, sc * P : (sc + 1) * P],
                        identity=ident[:E, :E],
                    )
                s4 = wpool.tile([P, 2], f32, tag="s4")
                nc.vector.reduce_sum(out=s4[:], in_=e4[:], axis=mybir.AxisListType.X)
                r4 = wpool.tile([P, 2], f32, tag="r4")
                nc.vector.reciprocal(r4[:], s4[:])
                c4 = wpool.tile([P, 2 * E], f32, tag="c4")
                for sc in range(2):
                    e_ap, r_ap = broadcast_tensor_aps(
                        e4[:, sc, :], r4[:, sc : sc + 1]
                    )
                    nc.vector.tensor_mul(c4[:, sc * E : (sc + 1) * E], e_ap, r_ap)
                ct_ps = tpool.tile([2 * E, P], f32, tag="ctp")
                nc.tensor.transpose(out=ct_ps[:], in_=c4[:], identity=ident[:, :])
                ct_sb = wpool.tile([2 * E, P], f32, tag="ctsb")
                nc.vector.tensor_copy(ct_sb[:], ct_ps[:])
                nc.sync.dma_start(out=ct_r[:, ch, :], in_=ct_sb[:])
    split_excess_waits(nc)
    return nc


def build_expert_kernel():
    """Per core: one expert. Resident fp8 weight planes, fp8 DoubleRow
    grouped GEMM over pre-gathered/packed token planes, bias via K=1
    ones-matmul PSUM seed, prob scaling on the ACT PSUM->SBUF copy.
    Out: compact y [NM, P, O] bf16."""
    nc = bass.Bass()
    xt = nc.dram_tensor("xt", [NM, 2, P, D], f8, kind="ExternalInput")
    w = nc.dram_tensor("w", [2, NJ, P, 2, O], f8, kind="ExternalInput")
    bias = nc.dram_tensor("bias", [P, O], bf16, kind="ExternalInput")
    prob = nc.dram_tensor("prob", [P, NM], f32, kind="ExternalInput")
    y = nc.dram_tensor("y", [NM, P, O], bf16, kind="ExternalOutput")

    xt_r = xt.rearrange("m pl p (j i f) -> p m pl j i f", j=NJ, i=2)
    w_r = w.rearrange("pl j p i o -> p pl j i o")
    y_r = y.rearrange("m p o -> p m o")

    with TileContext(nc) as tc:
        with (
            tc.tile_pool(name="const", bufs=1) as cpool,
            tc.tile_pool(name="wts", bufs=1) as wtpool,
            tc.tile_pool(name="xin", bufs=3) as xpool,
            tc.tile_pool(name="yout", bufs=2) as ypool,
            tc.tile_pool(name="psy", bufs=2, space="PSUM") as ppool,
        ):
            bias_sb = cpool.tile([P, O], bf16)
            nc.sync.dma_start(out=bias_sb[:], in_=bias[:, :])
            prob_sb = cpool.tile([P, NM], f32)
            nc.sync.dma_start(out=prob_sb[:], in_=prob[:, :])

            wt = [[None] * NJ for _ in range(2)]

            def load_w(pl, j):
                t = wtpool.tile([P, 2, O], f8, tag=f"w{pl}_{j}", name=f"w{pl}_{j}")
                nc.sync.dma_start(out=t[:], in_=w_r[:, pl, j])
                wt[pl][j] = t

            def load_x(m):
                t = xpool.tile([P, 2, NJ, 2, P], f8, tag="xt")
                nc.sync.dma_start(out=t[:], in_=xt_r[:, m])
                return t

            # first x tile + j=0 weight planes up front, then the rest
            xts = [load_x(0)]
            load_w(0, 0)
            load_w(1, 0)
            xts.append(load_x(1))
            for j in range(1, NJ):
                load_w(0, j)
                load_w(1, j)

            for m in range(NM):
                if m + 2 < NM:
                    xts.append(load_x(m + 2))
                xtile = xts[m]
                ps = [
                    ppool.tile([P, 512], f32, tag=f"ps{c}", name=f"ps{c}")
                    for c in range(4)
                ]
                for j in range(NJ):
                    for xp, wp in ((0, 0), (0, 1), (1, 0)):
                        first = j == 0 and xp == 0 and wp == 0
                        last = j == NJ - 1 and xp == 1
                        for c in range(4):
                            nc.tensor.matmul(
                                ps[c][:],
                                lhsT=xtile[:, xp, j],
                                rhs=wt[wp][j][:, :, c * 512 : (c + 1) * 512],
                                start=first, stop=last,
                                perf_mode=DR,
                            )
                tsb = ypool.tile([P, O], f32, tag="tsb")
                ysb = ypool.tile([P, O], bf16, tag="ysb")
                for c in range(4):
                    sl = slice(c * 512, (c + 1) * 512)
                    nc.vector.tensor_add(tsb[:, sl], ps[c][:], bias_sb[:, sl])
                    nc.scalar.activation(
                        ysb[:, sl], tsb[:, sl],
                        mybir.ActivationFunctionType.Copy,
                        scale=prob_sb[:, m : m + 1],
                    )
                nc.sync.dma_start(out=y_r[:, m], in_=ysb[:])
    split_excess_waits(nc)
    return nc


_gate_nc = None
_exp_nc = None


def kernel(x, W_e, b_e, W_g, b_g):
    global _gate_nc, _exp_nc
    x = np.ascontiguousarray(np.asarray(x, dtype=np.float32))
    W_e = np.asarray(W_e, dtype=np.float32)
    b_e = np.asarray(b_e, dtype=np.float32)
    W_g = np.asarray(W_g, dtype=np.float32)
    b_g = np.asarray(b_g, dtype=np.float32)

    # ---- Launch A: gate ----
    xT = np.ascontiguousarray(x.T)  # [D, B]
    xhi = xT.astype(BF16)
    xlo = (xT - xhi.astype(np.float32)).astype(BF16)
    wghi = W_g.astype(BF16)
    wglo = (W_g - wghi.astype(np.float32)).astype(BF16)
    wgh_d = np.ascontiguousarray(wghi.reshape(NKT, P, E))
    wgl_d = np.ascontiguousarray(wglo.reshape(NKT, P, E))
    bg_d = b_g.reshape(E, 1)

    if _gate_nc is None:
        _gate_nc = build_gate_kernel()
    in_maps = [
        {
            "xh": np.ascontiguousarray(
                xhi[:, i * BS : (i + 1) * BS].reshape(NKT, P, BS)
            ),
            "xl": np.ascontiguousarray(
                xlo[:, i * BS : (i + 1) * BS].reshape(NKT, P, BS)
            ),
            "wgh": wgh_d,
            "wgl": wgl_d,
            "bg": bg_d,
        }
        for i in range(E)
    ]
    res_a = run_bass_kernel_spmd(_gate_nc, in_maps, core_ids=list(range(8)))
    # ct [NCH, 2E, P] -> probs [BS, E] per core
    probs = np.concatenate(
        [
            r["ct"].reshape(NCH, 2, E, P).transpose(0, 1, 3, 2).reshape(BS, E)
            for r in res_a.results
        ],
        axis=0,
    )  # [B, E]

    # ---- Host routing bookkeeping ----
    top2 = np.argsort(-probs, axis=1, kind="stable")[:, :2]  # ties -> lower idx
    p2 = np.take_along_axis(probs, top2, axis=1)
    c_full = np.zeros_like(probs)
    np.put_along_axis(c_full, top2, p2, axis=1)

    # fp8 planes of x (computed once, rows gathered per expert)
    x0 = x.astype(FP8)
    x2 = (x - x0.astype(np.float32)).astype(FP8)

    idx_list, prob_list, n_list = [], [], []
    for e in range(E):
        sel = np.nonzero(c_full[:, e] > 0.0)[0].astype(np.int32)
        n = len(sel)
        assert n <= C, f"expert {e} over capacity: {n} > {C}"
        idxp = np.zeros(C, np.int32)
        idxp[:n] = sel
        probp = np.zeros(C, np.float32)
        probp[:n] = c_full[sel, e]
        idx_list.append(idxp)
        prob_list.append(np.ascontiguousarray((probp / SW).reshape(NM, P).T))
        n_list.append(n)

    def pack_x(plane, idxp):
        g = plane[idxp]  # [C, D] fp8
        return g.reshape(NM, P, NJ, 2, P).transpose(0, 4, 2, 3, 1)

    def pack_w(Wf):
        # [D, O] float -> [NJ, P, 2, O] fp8 plane pair
        Wp = np.clip(Wf * SW, -240, 240)
        w0 = Wp.astype(FP8)
        w1 = np.clip(Wp - w0.astype(np.float32), -240, 240).astype(FP8)
        return (
            w0.reshape(NJ, 2, P, O).transpose(0, 2, 1, 3),
            w1.reshape(NJ, 2, P, O).transpose(0, 2, 1, 3),
        )

    if _exp_nc is None:
        _exp_nc = build_expert_kernel()
    in_maps = []
    for e in range(E):
        xt_d = np.empty((NM, 2, P, D), FP8)
        xt_d[:, 0] = pack_x(x0, idx_list[e]).reshape(NM, P, D)
        xt_d[:, 1] = pack_x(x2, idx_list[e]).reshape(NM, P, D)
        w0_d, w1_d = pack_w(W_e[e])
        w_d = np.empty((2, NJ, P, 2, O), FP8)
        w_d[0] = w0_d
        w_d[1] = w1_d
        in_maps.append(
            {
                "xt": xt_d,
                "w": w_d,
                "bias": np.ascontiguousarray(
                    np.broadcast_to((b_e[e] * SW).astype(BF16).reshape(1, O), (P, O))
                ),
                "prob": prob_list[e],
            }
        )
    res_b = run_bass_kernel_spmd(_exp_nc, in_maps, core_ids=list(range(8)))

    out = np.zeros((B, O), np.float32)
    for e in range(E):
        n = n_list[e]
        ye = res_b.results[e]["y"].reshape(C, O)[:n].astype(np.float32)
        out[idx_list[e][:n]] += ye
    return out
